# revision 3
# baseline (speedup 1.0000x reference)
"""Causal single-head attention on 8 Trainium2 NeuronCores.

Reference computation (per batch b of 16):
    q = x @ Wq; k = x @ Wk; v = x @ Wv        # x [2048, 512], W* [512, 64]
    out = softmax_causal(q @ k.T / 8) @ v     # out [2048, 64]

Sharding: data-parallel over batch, 2 batches per core, weights replicated.

Per-core kernel (batch-local b in {0,1}), bf16 matmul inputs with fp32
PSUM accumulation:
  - PE warmup: dummy matmuls at t=0 absorb the tensor-engine p-state ramp
    while the first x chunk is still in flight
  - x arrives chunk-major ([128, NQ, ND, 512] per batch) so every chunk is
    ONE contiguous 2D DMA; few DMA instructions keeps the descriptor
    prefetch prologue short
  - qT/kT: psum[0:64]=qT, psum[64:128]=kT via packed lhsT [Wq|Wk]; merged
    SBUF shift DMAs provide qT at partitions 64:128 (qd) and kT at 0:64
    (klo) for the row-tiled score pairs
  - v projection col-tiled across the two batches, then PE transposes into
    v natural, packed v1[., j, .] = [v_j | 1] (ones column makes PV emit
    the softmax denominator for free)
  - attention processes BOTH batches' chunks interleaved: per slot the PE
    runs 4 score matmuls back-to-back then 4 PV matmuls back-to-back
    (same-geometry grouping hides LDWEIGHTS); softmax runs on BOTH
    elementwise engines (ACT exp for first-half blocks, DVE fused
    Schraudolph-exp + causal-mask scalar_tensor_tensor for second-half)
  - causal: k-blocks above the diagonal skipped; diagonal blocks get
    suffix-sliced matmuls (plus a mask multiply for chunk-0 first halves)
  - oT[65, 512] accumulates [v|1].T @ p~ per chunk in psum; row 64 is the
    denominator l; the host does out = o / l (+ final transpose)
"""

import sys

sys.path.insert(0, "/opt/trn_rl_repo")

import numpy as np
import ml_dtypes

B, T, D, HD = 16, 2048, 512, 64
NCORES = 8
BPC = B // NCORES          # batches per core
NQ = T // 512              # 512-wide q chunks per batch
NJ = T // 128              # 128-wide k blocks per batch
ND = D // 128              # 128-deep contraction tiles

SCH_A = 16.0 * np.log2(np.e)           # 128 * log2(e) * (1/sqrt(HD))
SCH_B = 16256.0 - 128.0 * 0.045       # 127*2^7 minus Schraudolph centering

_cache = {}


def _build_nc():
    import concourse.bacc as bacc
    import concourse.mybir as mybir
    import concourse.tile as tile

    F32 = mybir.dt.float32
    BF16 = mybir.dt.bfloat16
    I16 = mybir.dt.int16
    AF = mybir.ActivationFunctionType
    ALU = mybir.AluOpType

    nc = bacc.Bacc("TRN2", target_bir_lowering=False, debug=False)

    # x chunk-major: xt[b, p, ((q*ND)+d)*512 + u] = x[b, 512q+u, 128d+p]
    xt_d = nc.dram_tensor("xt", [BPC, 128, NQ * ND * 512], BF16,
                          kind="ExternalInput")
    # per d-tile: cols 0:128 = [Wq|Wk], 128:192 = Wv
    w_d = nc.dram_tensor("w", [128, ND * 192], BF16, kind="ExternalInput")
    # cols 0:64 = stacked identity (two 64x64 eyes), 64:192 = causal mask
    cst_d = nc.dram_tensor("cst", [128, 192], BF16, kind="ExternalInput")
    # Schraudolph bias plane: C[k, u] = SCH_B - 10000*(k > u); columns
    # 512:1024 are all-SCH_B (the non-masked case)
    schc_d = nc.dram_tensor("schc", [128, 1024], F32, kind="ExternalInput")
    ot_d = nc.dram_tensor("ot", [BPC, HD + 1, T], F32, kind="ExternalOutput")

    with tile.TileContext(nc) as tc:
        with (
            tc.tile_pool(name="const", bufs=1) as cpool,
            tc.tile_pool(name="xt", bufs=1) as xtpool,
            tc.tile_pool(name="qk", bufs=1) as qkpool,
            tc.tile_pool(name="qd", bufs=1) as qdpool,
            tc.tile_pool(name="klo", bufs=1) as klopool,
            tc.tile_pool(name="vt", bufs=1) as vtpool,
            tc.tile_pool(name="v1", bufs=1) as v1pool,
            tc.tile_pool(name="pt", bufs=2) as ptpool,
            tc.tile_pool(name="ob", bufs=2) as obpool,
            tc.tile_pool(name="st", bufs=1, space="PSUM") as stpool,
            tc.tile_pool(name="otp", bufs=1, space="PSUM") as otpool,
            tc.tile_pool(name="aux", bufs=2, space="PSUM") as auxpool,
        ):
            # ---- t=0 engine warm-up (no DMA dependencies) ----
            # PE p-state ramp: dummy matmuls while the first x chunk flies
            dum = cpool.tile([128, 128], BF16, tag="dum")
            nc.vector.memset(dum[:], 0.0)
            dpsum = auxpool.tile([128, 128], F32, tag="aux", name="dpsum")
            for _ in range(20):
                nc.tensor.matmul(dpsum[:], dum[:], dum[:],
                                 start=True, stop=True)
            # warm the exp table set on ACT
            scratch = cpool.tile([1, 8], F32, tag="scratch")
            nc.vector.memset(scratch[:], 0.0)
            scratch2 = cpool.tile([1, 8], F32, tag="scratch2")
            nc.scalar.activation(scratch2[:], scratch[:], AF.Exp)

            # ---- input DMAs: few, contiguous, ordered by need ----
            xtc = {}
            for b in range(BPC):
                xtc[b] = xtpool.tile([128, NQ, ND, 512], BF16,
                                     tag=f"xt{b}", name=f"xt{b}")
            xsrc = {
                b: xt_d[b].rearrange("p (q d u) -> p q d u", q=NQ, d=ND)
                for b in range(BPC)
            }
            w = cpool.tile([128, ND, 192], BF16, tag="w")
            nc.sync.dma_start(w[:], w_d[:].rearrange("p (d c) -> p d c", d=ND))
            cst = cpool.tile([128, 192], BF16, tag="cst")
            nc.scalar.dma_start(cst[:], cst_d[:])
            nc.sync.dma_start(xtc[0][:, 0], xsrc[0][:, 0])
            nc.scalar.dma_start(xtc[1][:, 0], xsrc[1][:, 0])
            schc = cpool.tile([128, 1024], F32, tag="schc", name="schc")
            nc.scalar.dma_start(schc[:], schc_d[:])
            for Q in range(1, NQ):
                nc.sync.dma_start(xtc[0][:, Q], xsrc[0][:, Q])
                nc.scalar.dma_start(xtc[1][:, Q], xsrc[1][:, Q])
            ident = cst[:, 0:64]
            mask = cst[:, 64:192]

            qks, qds, klos, v1s = {}, {}, {}, {}
            for b in range(BPC):
                qks[b] = qkpool.tile([128, T], BF16, tag=f"qk{b}", name=f"qk{b}")
                qds[b] = qdpool.tile([128, T], BF16, tag=f"qd{b}", name=f"qd{b}")
                klos[b] = klopool.tile([64, 1024], BF16, tag=f"klo{b}",
                                       name=f"klo{b}")
                v1s[b] = v1pool.tile([128, NJ, HD + 1], BF16, tag=f"v1{b}",
                                     name=f"v1{b}")
                nc.vector.memset(v1s[b][:, :, HD:HD + 1], 1.0)
            vt2 = vtpool.tile([128, T], BF16, tag="vt", name="vt2")

            def emit_qkproj(b, Q):
                """qT (partitions 0:64) / kT (64:128) for tokens
                [512Q, 512Q+512) of batch b via packed lhsT [Wq|Wk]."""
                s = slice(512 * Q, 512 * (Q + 1))
                p = auxpool.tile([128, 512], F32, tag="aux", name="pqk")
                for d in range(ND):
                    nc.tensor.matmul(
                        p[:], w[:, d, 0:128], xtc[b][:, Q, d, :],
                        start=(d == 0), stop=(d == ND - 1),
                    )
                if b == 0:
                    nc.scalar.copy(qks[b][:, s], p[:])
                else:
                    nc.vector.tensor_copy(qks[b][:, s], p[:])

            def emit_shifts(b, hi):
                """Merged SBUF partition-shift DMAs: qT down to partitions
                64:128 (qd) for the whole half, kT up to 0:64 (klo) for
                tokens 0:1024."""
                if not hi:
                    nc.sync.dma_start(qds[b][64:128, 0:1024],
                                      qks[b][0:64, 0:1024])
                    nc.scalar.dma_start(klos[b][0:64, :],
                                        qks[b][64:128, 0:1024])
                else:
                    nc.sync.dma_start(qds[b][64:128, 1024:2048],
                                      qks[b][0:64, 1024:2048])

            def emit_vpair(Q):
                """v for tokens [512Q, 512Q+512) of BOTH batches: projection
                col-tiled (b0 -> psum rows 0:64, b1 -> rows 64:128), PE
                transposes row-tiled, both pairs running concurrently."""
                s = slice(512 * Q, 512 * (Q + 1))
                pvv = auxpool.tile([128, 512], F32, tag="aux", name="pvv")
                for d in range(ND):
                    nc.tensor.matmul(
                        pvv[0:64, :], w[:, d, 128:192], xtc[0][:, Q, d, :],
                        start=(d == 0), stop=(d == ND - 1),
                    )
                    nc.tensor.matmul(
                        pvv[64:128, :], w[:, d, 128:192], xtc[1][:, Q, d, :],
                        start=(d == 0), stop=(d == ND - 1),
                    )
                nc.scalar.copy(vt2[:, s], pvv[:])
                for t2 in range(2 * Q, 2 * Q + 2):
                    p2a = auxpool.tile([128, 128], BF16, tag="aux", name="p2a")
                    p2b = auxpool.tile([128, 128], BF16, tag="aux", name="p2b")
                    for tt in range(2):
                        ts_ = slice(128 * (2 * t2 + tt), 128 * (2 * t2 + tt + 1))
                        nc.tensor.transpose(
                            p2a[:, 64 * tt:64 * (tt + 1)],
                            vt2[0:64, ts_], ident[0:64, :],
                        )
                        nc.tensor.transpose(
                            p2b[:, 64 * tt:64 * (tt + 1)],
                            vt2[64:128, ts_], ident[64:128, :],
                        )
                    nc.vector.tensor_copy(
                        v1s[0][:, 2 * t2:2 * t2 + 2, 0:HD],
                        p2a[:].rearrange("p (a c) -> p a c", a=2),
                    )
                    nc.vector.tensor_copy(
                        v1s[1][:, 2 * t2:2 * t2 + 2, 0:HD],
                        p2b[:].rearrange("p (a c) -> p a c", a=2),
                    )

            def emit_attn_pair(Q):
                """One query chunk for BOTH batches, group-interleaved.
                Per slot the PE runs 4 score matmuls back-to-back (row-tiled
                pairs x 2 batches) then 4 PV matmuls back-to-back (prev
                slot), so same-geometry runs hide LDWEIGHTS. Softmax per
                batch: ACT exps block j1, DVE runs the fused Schraudolph-exp
                + causal-mask scalar_tensor_tensor for block j2."""
                njb = 4 * (Q + 1)          # causal k-blocks for this chunk
                half = njb // 2
                jlast = njb - 1
                pots = {}
                for b in range(BPC):
                    pots[b] = otpool.tile([HD + 1, 512], F32, tag=f"ot{b}",
                                          name=f"pot{b}")
                pending = {0: None, 1: None}

                def w0_of(j):
                    return 128 * (j - 4 * Q) if j >= 4 * Q else 0

                def emit_pv(b, pa, pb, j1, j2):
                    for p_tile, j in ((pa, j1), (pb, j2)):
                        w0 = w0_of(j)
                        nc.tensor.matmul(
                            pots[b][:, w0:512],
                            v1s[b][:, j, :],
                            p_tile[:, w0:512],
                            start=(j == 0),
                            stop=(j == jlast),
                        )

                for g in range(half):
                    j1, j2 = g, half + g
                    w1, w2 = w0_of(j1), w0_of(j2)
                    sts = {}
                    for b in range(BPC):
                        psa = stpool.tile([128, 512], F32, tag=f"sta{b}",
                                          name="psa")
                        psb = stpool.tile([128, 512], F32, tag=f"stb{b}",
                                          name="psb")
                        nc.tensor.matmul(
                            psa[:, w1:512],
                            klos[b][0:64, 128 * j1:128 * (j1 + 1)],
                            qks[b][0:64, 512 * Q + w1:512 * (Q + 1)],
                            start=True, stop=True,
                        )
                        nc.tensor.matmul(
                            psb[:, w2:512],
                            qks[b][64:128, 128 * j2:128 * (j2 + 1)],
                            qds[b][64:128, 512 * Q + w2:512 * (Q + 1)],
                            start=True, stop=True,
                        )
                        sts[b] = (psa, psb)
                    pts = {}
                    for b in range(BPC):
                        psa, psb = sts[b]
                        pta = ptpool.tile([128, 512], BF16, tag=f"pta{b}",
                                          name="pta")
                        ptb = ptpool.tile([128, 512], BF16, tag=f"ptb{b}",
                                          name="ptb")
                        nc.scalar.activation(
                            pta[:, w1:512], psa[:, w1:512], AF.Exp,
                            scale=1.0 / np.sqrt(HD),
                        )
                        if j1 >= 4 * Q:  # only chunk 0: diagonal in idx0
                            nc.vector.tensor_mul(
                                pta[:, w1:w1 + 128], pta[:, w1:w1 + 128],
                                mask[:, 0:128],
                            )
                        # fused Schraudolph exp + causal mask: the bias plane
                        # is SCH_B - 10000 in masked spots -> bf16 ~ 0
                        u0 = 0 if j2 >= 4 * Q else 512
                        nc.vector.scalar_tensor_tensor(
                            ptb[:, w2:512].bitcast(I16),
                            psb[:, w2:512],
                            SCH_A,
                            schc[:, u0:u0 + 512 - w2],
                            ALU.mult, ALU.add,
                        )
                        pts[b] = (pta, ptb)
                    for b in range(BPC):
                        if pending[b] is not None:
                            emit_pv(b, *pending[b])
                        pending[b] = (pts[b][0], pts[b][1], j1, j2)
                for b in range(BPC):
                    emit_pv(b, *pending[b])
                    # unnormalized o (rows 0:64) + denominator l (row 64)
                    # out; the host divides
                    osb = obpool.tile([HD + 1, 512], F32, tag=f"ob{b}",
                                      name=f"osb{b}")
                    if b == 0:
                        nc.scalar.copy(osb[:], pots[b][:])
                        nc.sync.dma_start(
                            ot_d[b, :, 512 * Q:512 * (Q + 1)], osb[:])
                    else:
                        nc.vector.tensor_copy(osb[:], pots[b][:])
                        nc.scalar.dma_start(
                            ot_d[b, :, 512 * Q:512 * (Q + 1)], osb[:])

            # ---- emission schedule: projections pipelined ahead of the
            # chunk-interleaved attention; ends on the small chunk 0 ----
            emit_qkproj(0, 0)
            emit_qkproj(1, 0)
            emit_vpair(0)
            emit_qkproj(0, 1)
            emit_qkproj(1, 1)
            emit_shifts(0, False)
            emit_shifts(1, False)
            emit_vpair(1)
            emit_attn_pair(0)
            emit_qkproj(0, 2)
            emit_qkproj(1, 2)
            emit_vpair(2)
            emit_attn_pair(1)
            emit_qkproj(0, 3)
            emit_qkproj(1, 3)
            emit_shifts(0, True)
            emit_shifts(1, True)
            emit_vpair(3)
            emit_attn_pair(2)
            emit_attn_pair(3)

    nc.compile()
    return nc


def _get_nc():
    if "nc" not in _cache:
        _cache["nc"] = _build_nc()
    return _cache["nc"]


def _pack_w(w):
    # [512, C] -> partition-major [128, ND, C]: out[p, d, c] = w[128d+p, c]
    return w.reshape(ND, 128, -1).transpose(1, 0, 2)


def kernel(x, Wq, Wk, Wv, _trace=False, _trace_kwargs=None):
    from concourse.bass_utils import run_bass_kernel_spmd

    x = np.asarray(x, dtype=np.float32)
    Wq = np.asarray(Wq, dtype=np.float32)
    Wk = np.asarray(Wk, dtype=np.float32)
    Wv = np.asarray(Wv, dtype=np.float32)

    nc = _get_nc()

    bf16 = ml_dtypes.bfloat16
    w = np.ascontiguousarray(
        _pack_w(np.concatenate([Wq, Wk, Wv], axis=1)).reshape(128, ND * 192)
    ).astype(bf16)
    eye = np.eye(64, dtype=np.float32)
    ident = np.concatenate([eye, eye], axis=0)
    mask = np.triu(np.ones((128, 128), dtype=np.float32))
    cst = np.concatenate([ident, mask], axis=1).astype(bf16)
    kk, uu = np.meshgrid(np.arange(128), np.arange(1024), indexing="ij")
    schc = (SCH_B - 10000.0 * (kk > uu)).astype(np.float32)

    in_maps = []
    for c in range(NCORES):
        xs = []
        for b in range(BPC):
            xb = x[BPC * c + b].T  # [D, T]
            xb = xb.reshape(ND, 128, NQ, 512).transpose(1, 2, 0, 3)
            xs.append(xb.reshape(128, NQ * ND * 512))
        xt = np.ascontiguousarray(np.stack(xs)).astype(bf16)
        in_maps.append({"xt": xt, "w": w, "cst": cst, "schc": schc})

    kwargs = dict(_trace_kwargs or {})
    res = run_bass_kernel_spmd(
        nc, in_maps, list(range(NCORES)), trace=_trace, **kwargs
    )

    out = np.empty((B, T, HD), dtype=np.float32)
    for c in range(NCORES):
        ot = res.results[c]["ot"]  # [BPC, HD+1, T] unnormalized + denominator
        o = ot[:, 0:HD, :] / ot[:, HD:HD + 1, :]
        out[BPC * c:BPC * (c + 1)] = o.transpose(0, 2, 1)
    if _trace:
        _cache["last_results"] = res
    return out


# revision 16
# speedup vs baseline: 1.1264x; 1.1264x over previous
"""Causal single-head attention on 8 Trainium2 NeuronCores.

Reference computation (per batch b of 16):
    q = x @ Wq; k = x @ Wk; v = x @ Wv        # x [2048, 512], W* [512, 64]
    out = softmax_causal(q @ k.T / 8) @ v     # out [2048, 64]

Sharding: data-parallel over batch, 2 batches per core, weights replicated.

Per-core kernel (batch-local b in {0,1}), bf16 matmul inputs with fp32
PSUM accumulation:
  - PE warmup: dummy matmuls at t=0 absorb the tensor-engine p-state ramp
    while the first x chunk is still in flight
  - x arrives chunk-major ([128, NQ, ND, 512] per batch) so every chunk is
    ONE contiguous 2D DMA; few DMA instructions keeps the descriptor
    prefetch prologue short
  - qT/kT: psum[0:64]=qT, psum[64:128]=kT via packed lhsT [Wq|Wk]; merged
    SBUF shift DMAs provide qT at partitions 64:128 (qd) and kT at 0:64
    (klo) for the row-tiled score pairs
  - v projection col-tiled across the two batches, then PE transposes into
    v natural, packed v1[., j, .] = [v_j | 1] (ones column makes PV emit
    the softmax denominator for free)
  - attention processes BOTH batches' chunks interleaved: per slot the PE
    runs 4 score matmuls back-to-back then 4 PV matmuls back-to-back
    (same-geometry grouping hides LDWEIGHTS); softmax runs on BOTH
    elementwise engines (ACT exp for first-half blocks, DVE fused
    Schraudolph-exp + causal-mask scalar_tensor_tensor for second-half)
  - causal: k-blocks above the diagonal skipped; diagonal blocks get
    suffix-sliced matmuls (plus a mask multiply for chunk-0 first halves)
  - oT[65, 512] accumulates [v|1].T @ p~ per chunk in psum; row 64 is the
    denominator l; the host does out = o / l (+ final transpose)
"""

import sys

sys.path.insert(0, "/opt/trn_rl_repo")

import numpy as np
import ml_dtypes

B, T, D, HD = 16, 2048, 512, 64
NCORES = 8
BPC = B // NCORES          # batches per core
NQ = T // 512              # 512-wide q chunks per batch
NJ = T // 128              # 128-wide k blocks per batch
ND = D // 128              # 128-deep contraction tiles

SCH_A = 16.0 * np.log2(np.e)           # 128 * log2(e) * (1/sqrt(HD))
SCH_B = 16256.0 - 128.0 * 0.045       # 127*2^7 minus Schraudolph centering

_cache = {}


def _build_nc():
    import concourse.bacc as bacc
    import concourse.mybir as mybir
    import concourse.tile as tile

    F32 = mybir.dt.float32
    BF16 = mybir.dt.bfloat16
    I16 = mybir.dt.int16
    AF = mybir.ActivationFunctionType
    ALU = mybir.AluOpType

    nc = bacc.Bacc("TRN2", target_bir_lowering=False, debug=False)

    xt_d = nc.dram_tensor("xt", [BPC, D, T], BF16, kind="ExternalInput")
    # per d-tile: cols 0:128 = [Wq|Wk], 128:192 = Wv
    w_d = nc.dram_tensor("w", [128, ND * 192], BF16, kind="ExternalInput")
    # cols 0:64 = stacked identity (two 64x64 eyes), 64:192 = causal mask
    cst_d = nc.dram_tensor("cst", [128, 192], BF16, kind="ExternalInput")
    # Schraudolph bias plane: C[k, u] = SCH_B - 10000*(k > u); columns
    # 512:1024 are all-SCH_B (the non-masked case)
    schc_d = nc.dram_tensor("schc", [128, 1024], F32, kind="ExternalInput")
    ot_d = nc.dram_tensor("ot", [BPC, HD + 1, T], F32, kind="ExternalOutput")

    with tile.TileContext(nc) as tc:
        with (
            tc.tile_pool(name="const", bufs=1) as cpool,
            tc.tile_pool(name="xt", bufs=1) as xtpool,
            tc.tile_pool(name="qk", bufs=1) as qkpool,
            tc.tile_pool(name="qd", bufs=1) as qdpool,
            tc.tile_pool(name="klo", bufs=1) as klopool,
            tc.tile_pool(name="vt", bufs=1) as vtpool,
            tc.tile_pool(name="v1", bufs=1) as v1pool,
            tc.tile_pool(name="pt", bufs=2) as ptpool,
            tc.tile_pool(name="ob", bufs=2) as obpool,
            tc.tile_pool(name="st", bufs=1, space="PSUM") as stpool,
            tc.tile_pool(name="otp", bufs=1, space="PSUM") as otpool,
            tc.tile_pool(name="aux", bufs=2, space="PSUM") as auxpool,
        ):
            # ---- t=0 engine warm-up (no DMA dependencies) ----
            # PE p-state ramp: dummy matmuls while the first x chunk flies
            dum = cpool.tile([128, 512], BF16, tag="dum")
            nc.vector.memset(dum[:], 0.0)
            dpsum = auxpool.tile([128, 512], F32, tag="aux", name="dpsum")
            for _ in range(22):
                nc.tensor.matmul(dpsum[:], dum[:, 0:128], dum[:],
                                 start=True, stop=True)
            # warm the exp table set on ACT
            scratch = cpool.tile([1, 8], F32, tag="scratch")
            nc.vector.memset(scratch[:], 0.0)
            scratch2 = cpool.tile([1, 8], F32, tag="scratch2")
            nc.scalar.activation(scratch2[:], scratch[:], AF.Exp)

            # ---- input DMAs: few, contiguous, ordered by need ----
            xtc = {}
            for b in range(BPC):
                xtc[b] = xtpool.tile([128, ND, T], BF16,
                                     tag=f"xt{b}", name=f"xt{b}")
            xsrc = {
                b: xt_d[b].rearrange("(d p) t -> p d t", p=128)
                for b in range(BPC)
            }
            w = cpool.tile([128, ND, 192], BF16, tag="w")
            nc.sync.dma_start(w[:], w_d[:].rearrange("p (d c) -> p d c", d=ND))
            cst = cpool.tile([128, 192], BF16, tag="cst")
            nc.scalar.dma_start(cst[:], cst_d[:])
            nc.sync.dma_start(xtc[0][:, :, 0:512], xsrc[0][:, :, 0:512])
            nc.scalar.dma_start(xtc[1][:, :, 0:512], xsrc[1][:, :, 0:512])
            schc = cpool.tile([128, 1024], F32, tag="schc", name="schc")
            nc.scalar.dma_start(schc[:], schc_d[:])
            for lo, hi in ((512, 1024), (1024, T)):
                nc.sync.dma_start(xtc[0][:, :, lo:hi], xsrc[0][:, :, lo:hi])
                nc.scalar.dma_start(xtc[1][:, :, lo:hi], xsrc[1][:, :, lo:hi])
            ident = cst[:, 0:64]
            mask = cst[:, 64:192]

            qks, qds, klos, v1s = {}, {}, {}, {}
            for b in range(BPC):
                qks[b] = qkpool.tile([128, T], BF16, tag=f"qk{b}", name=f"qk{b}")
                qds[b] = qdpool.tile([128, T], BF16, tag=f"qd{b}", name=f"qd{b}")
                klos[b] = klopool.tile([64, 1024], BF16, tag=f"klo{b}",
                                       name=f"klo{b}")
                v1s[b] = v1pool.tile([128, NJ, HD + 1], BF16, tag=f"v1{b}",
                                     name=f"v1{b}")
                nc.vector.memset(v1s[b][:, :, HD:HD + 1], 1.0)
            vt2 = vtpool.tile([128, T], BF16, tag="vt", name="vt2")

            def emit_qkproj(b, Q):
                """qT (partitions 0:64) / kT (64:128) for tokens
                [512Q, 512Q+512) of batch b via packed lhsT [Wq|Wk]."""
                s = slice(512 * Q, 512 * (Q + 1))
                p = auxpool.tile([128, 512], F32, tag="aux", name="pqk")
                for d in range(ND):
                    nc.tensor.matmul(
                        p[:], w[:, d, 0:128], xtc[b][:, d, s],
                        start=(d == 0), stop=(d == ND - 1),
                    )
                if b == 0:
                    nc.scalar.copy(qks[b][:, s], p[:])
                else:
                    nc.vector.tensor_copy(qks[b][:, s], p[:])

            def emit_shifts(b, hi):
                """Merged SBUF partition-shift DMAs: qT down to partitions
                64:128 (qd) for the whole half, kT up to 0:64 (klo) for
                tokens 0:1024."""
                if not hi:
                    nc.sync.dma_start(qds[b][64:128, 0:1024],
                                      qks[b][0:64, 0:1024])
                    nc.scalar.dma_start(klos[b][0:64, :],
                                        qks[b][64:128, 0:1024])
                else:
                    nc.sync.dma_start(qds[b][64:128, 1024:2048],
                                      qks[b][0:64, 1024:2048])

            def emit_vpair(Q):
                """v for tokens [512Q, 512Q+512) of BOTH batches: projection
                col-tiled (b0 -> psum rows 0:64, b1 -> rows 64:128), PE
                transposes row-tiled, both pairs running concurrently."""
                s = slice(512 * Q, 512 * (Q + 1))
                pvv = auxpool.tile([128, 512], F32, tag="aux", name="pvv")
                for d in range(ND):
                    nc.tensor.matmul(
                        pvv[0:64, :], w[:, d, 128:192], xtc[0][:, d, s],
                        start=(d == 0), stop=(d == ND - 1),
                    )
                    nc.tensor.matmul(
                        pvv[64:128, :], w[:, d, 128:192], xtc[1][:, d, s],
                        start=(d == 0), stop=(d == ND - 1),
                    )
                nc.scalar.copy(vt2[:, s], pvv[:])
                for t2 in range(2 * Q, 2 * Q + 2):
                    p2a = auxpool.tile([128, 128], BF16, tag="aux", name="p2a")
                    p2b = auxpool.tile([128, 128], BF16, tag="aux", name="p2b")
                    for tt in range(2):
                        ts_ = slice(128 * (2 * t2 + tt), 128 * (2 * t2 + tt + 1))
                        nc.tensor.transpose(
                            p2a[:, 64 * tt:64 * (tt + 1)],
                            vt2[0:64, ts_], ident[0:64, :],
                        )
                        nc.tensor.transpose(
                            p2b[:, 64 * tt:64 * (tt + 1)],
                            vt2[64:128, ts_], ident[64:128, :],
                        )
                    nc.vector.tensor_copy(
                        v1s[0][:, 2 * t2:2 * t2 + 2, 0:HD],
                        p2a[:].rearrange("p (a c) -> p a c", a=2),
                    )
                    nc.vector.tensor_copy(
                        v1s[1][:, 2 * t2:2 * t2 + 2, 0:HD],
                        p2b[:].rearrange("p (a c) -> p a c", a=2),
                    )

            def emit_attn_pair(Q):
                """One query chunk for BOTH batches, group-interleaved.
                Per slot the PE runs 4 score matmuls back-to-back (row-tiled
                pairs x 2 batches) then 4 PV matmuls back-to-back (prev
                slot), so same-geometry runs hide LDWEIGHTS. Softmax per
                batch: ACT exps block j1, DVE runs the fused Schraudolph-exp
                + causal-mask scalar_tensor_tensor for block j2."""
                njb = 4 * (Q + 1)          # causal k-blocks for this chunk
                half = njb // 2
                jlast = njb - 1
                pots = {}
                for b in range(BPC):
                    pots[b] = otpool.tile([HD + 1, 512], F32, tag=f"ot{b}",
                                          name=f"pot{b}")
                pending = {0: None, 1: None}

                def w0_of(j):
                    return 128 * (j - 4 * Q) if j >= 4 * Q else 0

                def emit_pv(b, pa, pb, j1, j2):
                    for p_tile, j in ((pa, j1), (pb, j2)):
                        w0 = w0_of(j)
                        nc.tensor.matmul(
                            pots[b][:, w0:512],
                            v1s[b][:, j, :],
                            p_tile[:, w0:512],
                            start=(j == 0),
                            stop=(j == jlast),
                        )

                for g in range(half):
                    j1, j2 = g, half + g
                    w1, w2 = w0_of(j1), w0_of(j2)
                    sts = {}
                    for b in range(BPC):
                        psa = stpool.tile([128, 512], F32, tag=f"sta{b}",
                                          name="psa")
                        psb = stpool.tile([128, 512], F32, tag=f"stb{b}",
                                          name="psb")
                        nc.tensor.matmul(
                            psa[:, w1:512],
                            klos[b][0:64, 128 * j1:128 * (j1 + 1)],
                            qks[b][0:64, 512 * Q + w1:512 * (Q + 1)],
                            start=True, stop=True,
                        )
                        nc.tensor.matmul(
                            psb[:, w2:512],
                            qks[b][64:128, 128 * j2:128 * (j2 + 1)],
                            qds[b][64:128, 512 * Q + w2:512 * (Q + 1)],
                            start=True, stop=True,
                        )
                        sts[b] = (psa, psb)
                    pts = {}
                    for b in range(BPC):
                        psa, psb = sts[b]
                        pta = ptpool.tile([128, 512], BF16, tag=f"pta{b}",
                                          name="pta")
                        ptb = ptpool.tile([128, 512], BF16, tag=f"ptb{b}",
                                          name="ptb")
                        nc.scalar.activation(
                            pta[:, w1:512], psa[:, w1:512], AF.Exp,
                            scale=1.0 / np.sqrt(HD),
                        )
                        if j1 >= 4 * Q:  # only chunk 0: diagonal in idx0
                            nc.gpsimd.tensor_mul(
                                pta[:, w1:w1 + 128], pta[:, w1:w1 + 128],
                                mask[:, 0:128],
                            )
                        # fused Schraudolph exp + causal mask: the bias plane
                        # is SCH_B - 10000 in masked spots -> bf16 ~ 0
                        u0 = 0 if j2 >= 4 * Q else 512
                        nc.vector.scalar_tensor_tensor(
                            ptb[:, w2:512].bitcast(I16),
                            psb[:, w2:512],
                            SCH_A,
                            schc[:, u0:u0 + 512 - w2],
                            ALU.mult, ALU.add,
                        )
                        pts[b] = (pta, ptb)
                    for b in range(BPC):
                        if pending[b] is not None:
                            emit_pv(b, *pending[b])
                        pending[b] = (pts[b][0], pts[b][1], j1, j2)
                for b in range(BPC):
                    emit_pv(b, *pending[b])
                    # unnormalized o (rows 0:64) + denominator l (row 64)
                    # out; the host divides
                    osb = obpool.tile([HD + 1, 512], F32, tag=f"ob{b}",
                                      name=f"osb{b}")
                    if b == 0:
                        nc.scalar.copy(osb[:], pots[b][:])
                        nc.sync.dma_start(
                            ot_d[b, :, 512 * Q:512 * (Q + 1)], osb[:])
                    else:
                        nc.vector.tensor_copy(osb[:], pots[b][:])
                        nc.scalar.dma_start(
                            ot_d[b, :, 512 * Q:512 * (Q + 1)], osb[:])

            # ---- emission schedule: projections pipelined ahead of the
            # chunk-interleaved attention; ends on the small chunk 0 ----
            emit_qkproj(0, 0)
            emit_qkproj(1, 0)
            emit_vpair(0)
            emit_qkproj(0, 1)
            emit_qkproj(1, 1)
            emit_shifts(0, False)
            emit_shifts(1, False)
            emit_vpair(1)
            emit_attn_pair(0)
            emit_qkproj(0, 2)
            emit_qkproj(1, 2)
            emit_vpair(2)
            emit_attn_pair(1)
            emit_qkproj(0, 3)
            emit_qkproj(1, 3)
            emit_shifts(0, True)
            emit_shifts(1, True)
            emit_vpair(3)
            emit_attn_pair(2)
            emit_attn_pair(3)

    nc.compile()
    return nc


def _get_nc():
    if "nc" not in _cache:
        _cache["nc"] = _build_nc()
    return _cache["nc"]


def _pack_w(w):
    # [512, C] -> partition-major [128, ND, C]: out[p, d, c] = w[128d+p, c]
    return w.reshape(ND, 128, -1).transpose(1, 0, 2)


def kernel(x, Wq, Wk, Wv, _trace=False, _trace_kwargs=None):
    from concourse.bass_utils import run_bass_kernel_spmd

    x = np.asarray(x, dtype=np.float32)
    Wq = np.asarray(Wq, dtype=np.float32)
    Wk = np.asarray(Wk, dtype=np.float32)
    Wv = np.asarray(Wv, dtype=np.float32)

    nc = _get_nc()

    bf16 = ml_dtypes.bfloat16
    w = np.ascontiguousarray(
        _pack_w(np.concatenate([Wq, Wk, Wv], axis=1)).reshape(128, ND * 192)
    ).astype(bf16)
    eye = np.eye(64, dtype=np.float32)
    ident = np.concatenate([eye, eye], axis=0)
    mask = np.triu(np.ones((128, 128), dtype=np.float32))
    cst = np.concatenate([ident, mask], axis=1).astype(bf16)
    kk, uu = np.meshgrid(np.arange(128), np.arange(1024), indexing="ij")
    schc = (SCH_B - 10000.0 * (kk > uu)).astype(np.float32)

    in_maps = []
    for c in range(NCORES):
        xt = np.ascontiguousarray(
            x[BPC * c:BPC * (c + 1)].transpose(0, 2, 1).astype(bf16)
        )
        in_maps.append({"xt": xt, "w": w, "cst": cst, "schc": schc})

    kwargs = dict(_trace_kwargs or {})
    res = run_bass_kernel_spmd(
        nc, in_maps, list(range(NCORES)), trace=_trace, **kwargs
    )

    out = np.empty((B, T, HD), dtype=np.float32)
    for c in range(NCORES):
        ot = res.results[c]["ot"]  # [BPC, HD+1, T] unnormalized + denominator
        o = ot[:, 0:HD, :] / ot[:, HD:HD + 1, :]
        out[BPC * c:BPC * (c + 1)] = o.transpose(0, 2, 1)
    if _trace:
        _cache["last_results"] = res
    return out
